# revision 1
# baseline (speedup 1.0000x reference)
"""CrossAttention kernel for 8 TRN2 NeuronCores (head-parallel sharding).

Problem: x[2,2048,1024], context[2,2048,1024], 16 heads x 64 dim,
q/k/v projections + softmax attention + output projection.

Sharding: 2 heads per core (e-slice of 128 rows of Wq/Wk/Wv, 128 cols of Wo).
Each core computes a full-shape partial of the output projection for its
heads; the host sums the 8 partials and adds the bias.

All matmuls run as float32r (TF32-like, ~1e-4 rel err, full PE rate).
Layouts are chosen so no on-device transpose of x/context is needed:
the host passes x^T / context^T / W^T, and attention is computed as
S^T tiles ([key,query] layout) so softmax-sum folds into the PE via an
appended ones-column on V.
"""
import sys

sys.path.insert(0, "/opt/trn_rl_repo")

import numpy as np
from contextlib import ExitStack

import concourse.bass as bass  # noqa: F401
import concourse.tile as tile
from concourse import bacc, mybir
from concourse.bass_utils import run_bass_kernel_spmd
from concourse.masks import make_identity

B, N, M = 2, 2048, 2048
QDIM = 1024
HEADS = 16
DH = 64
INNER = 1024
NCORES = 8
ES = INNER // NCORES        # 128: e-slice (2 heads * 64) per core
SCALE = DH ** -0.5
T = B * N                   # 4096 query tokens; key tokens likewise B*M
KC = QDIM // 128            # 8 contraction chunks for the projections
MT = M // 128               # 16 key tiles per batch
F32 = mybir.dt.float32
F32R = mybir.dt.float32r
EXP = mybir.ActivationFunctionType.Exp


def build_nc(reps: int = 1):
    nc = bacc.Bacc("TRN2", target_bir_lowering=False, debug=False,
                   num_devices=NCORES)
    xT = nc.dram_tensor("xT", [QDIM, T], F32, kind="ExternalInput").ap()
    cT = nc.dram_tensor("cT", [QDIM, T], F32, kind="ExternalInput").ap()
    wqT = nc.dram_tensor("wqT", [QDIM, ES], F32, kind="ExternalInput").ap()
    wkT = nc.dram_tensor("wkT", [QDIM, ES], F32, kind="ExternalInput").ap()
    wvT = nc.dram_tensor("wvT", [QDIM, ES], F32, kind="ExternalInput").ap()
    woT = nc.dram_tensor("woT", [ES, QDIM], F32, kind="ExternalInput").ap()
    part = nc.dram_tensor("part", [T, QDIM], F32, kind="ExternalOutput").ap()

    xT3 = xT.rearrange("(kc p) n -> kc p n", p=128)
    cT3 = cT.rearrange("(kc p) n -> kc p n", p=128)

    with tile.TileContext(nc) as tc, ExitStack() as ctx:
        const = ctx.enter_context(tc.tile_pool(name="const", bufs=1))
        big = ctx.enter_context(tc.tile_pool(name="bigsb", bufs=1))
        xsl = ctx.enter_context(tc.tile_pool(name="xsl", bufs=4))
        epool = ctx.enter_context(tc.tile_pool(name="epool", bufs=4))
        opool = ctx.enter_context(tc.tile_pool(name="opool", bufs=2))
        bcp = ctx.enter_context(tc.tile_pool(name="bcp", bufs=2))
        outp = ctx.enter_context(tc.tile_pool(name="outp", bufs=3))
        psB = ctx.enter_context(tc.tile_pool(name="psB", bufs=2, space="PSUM"))
        psA = ctx.enter_context(tc.tile_pool(name="psA", bufs=2, space="PSUM"))

        ident_f = const.tile([128, 128], F32)
        make_identity(nc, ident_f[:])
        ident = const.tile([128, 128], F32R)
        nc.vector.tensor_copy(ident[:], ident_f[:])
        ones_f = const.tile([128, B * MT], F32)
        nc.vector.memset(ones_f[:], 1.0)
        onesR = const.tile([128, B * MT], F32R)
        nc.vector.tensor_copy(onesR[:], ones_f[:])
        wq_sb = const.tile([128, KC, ES], F32R)
        wk_sb = const.tile([128, KC, ES], F32R)
        wv_sb = const.tile([128, KC, ES], F32R)
        wo_sb = const.tile([128, QDIM], F32R)
        nc.gpsimd.dma_start(wq_sb[:], wqT.rearrange("(kc p) e -> p kc e", p=128))
        nc.gpsimd.dma_start(wk_sb[:], wkT.rearrange("(kc p) e -> p kc e", p=128))
        nc.gpsimd.dma_start(wv_sb[:], wvT.rearrange("(kc p) e -> p kc e", p=128))
        nc.gpsimd.dma_start(wo_sb[:], woT)

        for _rep in range(reps):
            QT = big.tile([128, T], F32R, tag="QT")
            KT = big.tile([128, T], F32R, tag="KT")
            VT = big.tile([128, T], F32R, tag="VT")
            vgA = big.tile([128, B * MT, DH + 1], F32R, tag="vgA")
            vgB = big.tile([128, B * MT, DH + 1], F32R, tag="vgB")
            nc.vector.tensor_copy(vgA[:, :, DH], onesR[:])
            nc.vector.tensor_copy(vgB[:, :, DH], onesR[:])
            ocats = {}

            def emit_qproj(b, nbp):
                col0 = b * N + nbp * 1024
                ps = psB.tile([128, 1024], F32, tag="ps")
                for k in range(KC):
                    xs = xsl.tile([128, 1024], F32R, tag="xs")
                    nc.gpsimd.dma_start(xs[:], xT3[k, :, col0:col0 + 1024])
                    for h in range(2):
                        sl = slice(h * 512, (h + 1) * 512)
                        nc.tensor.matmul(ps[:, sl], wq_sb[:, k, :], xs[:, sl],
                                         start=(k == 0), stop=(k == KC - 1))
                nc.vector.tensor_copy(QT[:, col0:col0 + 1024], ps[:])

            def emit_kvproj(b, nbp):
                col0 = b * N + nbp * 1024
                psk = psB.tile([128, 1024], F32, tag="ps")
                psv = psA.tile([128, 1024], F32, tag="pa")
                for k in range(KC):
                    cs = xsl.tile([128, 1024], F32R, tag="xs")
                    nc.gpsimd.dma_start(cs[:], cT3[k, :, col0:col0 + 1024])
                    for h in range(2):
                        sl = slice(h * 512, (h + 1) * 512)
                        nc.tensor.matmul(psk[:, sl], wk_sb[:, k, :], cs[:, sl],
                                         start=(k == 0), stop=(k == KC - 1))
                        nc.tensor.matmul(psv[:, sl], wv_sb[:, k, :], cs[:, sl],
                                         start=(k == 0), stop=(k == KC - 1))
                nc.vector.tensor_copy(KT[:, col0:col0 + 1024], psk[:])
                nc.vector.tensor_copy(VT[:, col0:col0 + 1024], psv[:])

            def emit_vtr(b, mt0, mt1):
                for mt in range(mt0, mt1):
                    g = b * MT + mt
                    mcol = b * N + mt * 128
                    for vg, base in ((vgA, 0), (vgB, DH)):
                        pt = psB.tile([128, DH], F32R, tag="ps")
                        nc.tensor.transpose(
                            pt[:], VT[base:base + DH, mcol:mcol + 128],
                            ident[base:base + DH, base:base + DH])
                        nc.vector.tensor_copy(vg[:, g, 0:DH], pt[:])

            def emit_attn(b, nhf):
                if b not in ocats:
                    ocats[b] = opool.tile([128, N], F32R, tag="oc",
                                          name=f"ocat_b{b}_{_rep}")
                ocat = ocats[b]
                qcol = b * N + nhf * 1024
                oA = psA.tile([128, 1024], F32, tag="pa")
                oB = psA.tile([128, 1024], F32, tag="pa")
                for mc in range(MT):
                    g = b * MT + mc
                    mcol = b * N + mc * 128
                    stA = psB.tile([128, 1024], F32, tag="ps")
                    stB = psB.tile([128, 1024], F32, tag="ps")
                    for nb in range(2):
                        sl = slice(nb * 512, (nb + 1) * 512)
                        qsl = slice(qcol + nb * 512, qcol + (nb + 1) * 512)
                        nc.tensor.matmul(stA[:, sl], KT[0:DH, mcol:mcol + 128],
                                         QT[0:DH, qsl], start=True, stop=True)
                        nc.tensor.matmul(stB[:, sl],
                                         KT[DH:2 * DH, mcol:mcol + 128],
                                         QT[DH:2 * DH, qsl],
                                         start=True, stop=True)
                    eA = epool.tile([128, 1024], F32R, tag="e")
                    eB = epool.tile([128, 1024], F32R, tag="e")
                    nc.scalar.activation(eA[:], stA[:], EXP, scale=SCALE)
                    nc.scalar.activation(eB[:], stB[:], EXP, scale=SCALE)
                    last = (mc == MT - 1)
                    for nb in range(2):
                        sl = slice(nb * 512, (nb + 1) * 512)
                        nc.tensor.matmul(oA[0:DH + 1, sl], vgA[:, g, :],
                                         eA[:, sl], start=(mc == 0), stop=last)
                        nc.tensor.matmul(oB[0:DH + 1, sl], vgB[:, g, :],
                                         eB[:, sl], start=(mc == 0), stop=last)
                for o_ps, row0 in ((oA, 0), (oB, DH)):
                    rr = bcp.tile([1, 1024], F32, tag="rr")
                    nc.vector.reciprocal(rr[:], o_ps[DH:DH + 1, 0:1024])
                    bc = bcp.tile([DH, 1024], F32, tag="bc")
                    nc.gpsimd.partition_broadcast(bc[:], rr[:])
                    nc.vector.tensor_mul(
                        ocat[row0:row0 + DH, nhf * 1024:(nhf + 1) * 1024],
                        o_ps[0:DH, 0:1024], bc[:])

            def emit_wo(b):
                ocat = ocats[b]
                for nt in range(N // 128):
                    po = psB.tile([128, 1024], F32, tag="ps")
                    for ob in range(2):
                        nc.tensor.matmul(po[:, ob * 512:(ob + 1) * 512],
                                         ocat[:, nt * 128:(nt + 1) * 128],
                                         wo_sb[:, ob * 512:(ob + 1) * 512],
                                         start=True, stop=True)
                    osb = outp.tile([128, 1024], F32, tag="os")
                    nc.vector.tensor_copy(osb[:], po[:])
                    nc.sync.dma_start(
                        part[b * N + nt * 128:b * N + (nt + 1) * 128, :], osb[:])

            # emission order chosen so attention(b=0) starts as soon as the
            # first K/V blocks land, and batch-1 projections / Wo(0) fill the
            # PE+DMA slack inside the ACT-paced attention stream.
            for b in range(B):
                emit_qproj(b, 0)
                emit_qproj(b, 1)
                emit_kvproj(b, 0)
                emit_kvproj(b, 1)
                emit_vtr(b, 0, MT)
                emit_attn(b, 0)
                emit_attn(b, 1)
                emit_wo(b)
    nc.compile()
    return nc


def make_in_maps(x, context, Wq, Wk, Wv, Wo):
    x = np.asarray(x, dtype=np.float32)
    context = np.asarray(context, dtype=np.float32)
    Wq = np.asarray(Wq, dtype=np.float32)
    Wk = np.asarray(Wk, dtype=np.float32)
    Wv = np.asarray(Wv, dtype=np.float32)
    Wo = np.asarray(Wo, dtype=np.float32)
    xT = np.ascontiguousarray(x.reshape(T, QDIM).T)
    cT = np.ascontiguousarray(context.reshape(T, QDIM).T)
    in_maps = []
    for c in range(NCORES):
        es = slice(c * ES, (c + 1) * ES)
        in_maps.append({
            "xT": xT,
            "cT": cT,
            "wqT": np.ascontiguousarray(Wq[es, :].T),
            "wkT": np.ascontiguousarray(Wk[es, :].T),
            "wvT": np.ascontiguousarray(Wv[es, :].T),
            "woT": np.ascontiguousarray(Wo[:, es].T),
        })
    return in_maps


_NC_CACHE = {}


def get_nc(reps: int = 1):
    if reps not in _NC_CACHE:
        _NC_CACHE[reps] = build_nc(reps)
    return _NC_CACHE[reps]


def run_on_hw(in_maps, reps: int = 1):
    nc = get_nc(reps)
    return run_bass_kernel_spmd(nc, in_maps, core_ids=list(range(NCORES)))


def kernel(x, context, Wq, Wk, Wv, Wo, bo):
    in_maps = make_in_maps(x, context, Wq, Wk, Wv, Wo)
    res = run_on_hw(in_maps, reps=1)
    acc = res.results[0]["part"].astype(np.float32).copy()
    for i in range(1, NCORES):
        acc += res.results[i]["part"]
    acc += np.asarray(bo, dtype=np.float32)[None, :]
    return acc.reshape(B, N, QDIM)



# revision 9
# speedup vs baseline: 1.5517x; 1.5517x over previous
"""CrossAttention kernel for 8 TRN2 NeuronCores (head-parallel sharding), v3.

Problem: x[2,2048,1024], context[2,2048,1024], 16 heads x 64 dim,
q/k/v projections + softmax attention + output projection.

Sharding: 2 heads per core (e-slice of 128 rows of Wq/Wk/Wv, 128 cols of Wo).
Each core computes a full-shape partial of the output projection for its
heads; the host sums the 8 partials and adds the bias.

Design notes:
- bf16 on-device datapath (host converts inputs); PSUM accumulation fp32.
- Inputs staged per batch into persistent SBUF tiles via 8 large HWDGE
  DMAs (512KB each) - minimizes SP-sequencer issue serialization.
- V^T tiles produced by one [128,128] DMA-transpose per key tile into
  vg2[ones | V_A | V_B | ones], so the softmax denominator folds into the
  attn*V matmuls as an extra lhsT column (output partition 0 / 64).
- Attention in 512-query blocks; both heads' S^T tiles land in one
  [128,1024] PSUM tile so exp runs as a single 1024-wide ACT instruction
  (ACT is the pacing engine at ~133us busy).
- The two heads' QK^T matmuls have 64-deep contraction and auto-place on
  disjoint PE row groups (tile_position (0,0)/(64,0)), overlapping on HW.
- Emission interleaves next-batch projections, Wo matmuls and output DMA
  into the ACT-paced attention stream.
"""
import sys

sys.path.insert(0, "/opt/trn_rl_repo")

import numpy as np
from contextlib import ExitStack

import concourse.bass as bass  # noqa: F401
import concourse.tile as tile
from concourse import bacc, mybir
from concourse.bass_utils import run_bass_kernel_spmd
from concourse.masks import make_identity

B, N, M = 2, 2048, 2048
QDIM = 1024
HEADS = 16
DH = 64
INNER = 1024
NCORES = 8
ES = INNER // NCORES        # 128: e-slice (2 heads * 64) per core
SCALE = DH ** -0.5
T = B * N                   # 4096 query tokens; key tokens likewise B*M
KC = QDIM // 128            # 8 contraction chunks for the projections
MT = M // 128               # 16 key tiles per batch
QB = 512                    # query block (1 PSUM bank of fp32)
NQB = N // QB               # 4 query blocks per batch
F32 = mybir.dt.float32
BF16 = mybir.dt.bfloat16
EXP = mybir.ActivationFunctionType.Exp


def build_nc(reps: int = 1):
    nc = bacc.Bacc("TRN2", target_bir_lowering=False, debug=False,
                   num_devices=NCORES)
    xT = nc.dram_tensor("xT", [QDIM, T], BF16, kind="ExternalInput").ap()
    cT = nc.dram_tensor("cT", [QDIM, T], BF16, kind="ExternalInput").ap()
    wqT = nc.dram_tensor("wqT", [QDIM, ES], BF16, kind="ExternalInput").ap()
    wkT = nc.dram_tensor("wkT", [QDIM, ES], BF16, kind="ExternalInput").ap()
    wvT = nc.dram_tensor("wvT", [QDIM, ES], BF16, kind="ExternalInput").ap()
    woT = nc.dram_tensor("woT", [ES, QDIM], BF16, kind="ExternalInput").ap()
    part = nc.dram_tensor("part", [T, QDIM], F32, kind="ExternalOutput").ap()

    xT3 = xT.rearrange("(kc p) n -> kc p n", p=128)
    cT3 = cT.rearrange("(kc p) n -> kc p n", p=128)

    with tile.TileContext(nc) as tc, ExitStack() as ctx:
        const = ctx.enter_context(tc.tile_pool(name="const", bufs=1))
        big = ctx.enter_context(tc.tile_pool(name="bigsb", bufs=1))
        epool = ctx.enter_context(tc.tile_pool(name="epool", bufs=4))
        opool = ctx.enter_context(tc.tile_pool(name="opool", bufs=2))
        bcp = ctx.enter_context(tc.tile_pool(name="bcp", bufs=4))
        outp = ctx.enter_context(tc.tile_pool(name="outp", bufs=3))
        psO = ctx.enter_context(tc.tile_pool(name="psO", bufs=2, space="PSUM"))
        psSt = ctx.enter_context(tc.tile_pool(name="psSt", bufs=2, space="PSUM"))
        psP = ctx.enter_context(tc.tile_pool(name="psP", bufs=2, space="PSUM"))

        ident_f = const.tile([128, 128], F32)
        make_identity(nc, ident_f[:])
        ident = const.tile([128, 128], BF16)
        nc.vector.tensor_copy(ident[:], ident_f[:])
        wq_sb = const.tile([128, KC, ES], BF16)
        wk_sb = const.tile([128, KC, ES], BF16)
        wv_sb = const.tile([128, KC, ES], BF16)
        wo_sb = const.tile([128, QDIM], BF16)
        nc.gpsimd.dma_start(wq_sb[:], wqT.rearrange("(kc p) e -> p kc e", p=128))
        nc.gpsimd.dma_start(wk_sb[:], wkT.rearrange("(kc p) e -> p kc e", p=128))
        nc.gpsimd.dma_start(wv_sb[:], wvT.rearrange("(kc p) e -> p kc e", p=128))
        nc.gpsimd.dma_start(wo_sb[:], woT)

        for _rep in range(reps):
            QT = big.tile([128, T], BF16, tag="QT")
            KT = big.tile([128, T], BF16, tag="KT")
            VT = big.tile([128, T], BF16, tag="VT")
            # vg2 columns: [V_A (64) | ones | V_B (64) | ones]
            vg2 = big.tile([128, B * MT, 2 * DH + 2], BF16, tag="vg2")
            nc.vector.memset(vg2[:, :, DH], 1.0)
            nc.vector.memset(vg2[:, :, 2 * DH + 1], 1.0)
            ocats = {}
            xcats = {}

            def load_piece(b, which):
                src3, tag = (xT3, "xcat") if which == "x" else (cT3, "ccat")
                cat = big.tile([128, KC, N], BF16, tag=tag,
                               name=f"{tag}_{b}_{_rep}")
                for k in range(KC):
                    nc.sync.dma_start(cat[:, k, :], src3[k, :, b * N:(b + 1) * N])
                xcats[(b, which)] = cat

            def q_piece(b, blk):
                xcat = xcats[(b, "x")]
                col0 = b * N + blk * QB
                ps = psP.tile([128, QB], F32, tag="pp")
                for k in range(KC):
                    nc.tensor.matmul(ps[:], wq_sb[:, k, :],
                                     xcat[:, k, blk * QB:(blk + 1) * QB],
                                     start=(k == 0), stop=(k == KC - 1))
                nc.vector.tensor_copy(QT[:, col0:col0 + QB], ps[:])

            def kv_piece(b, blk):
                ccat = xcats[(b, "c")]
                col0 = b * N + blk * QB
                psk = psP.tile([128, QB], F32, tag="pp")
                psv = psP.tile([128, QB], F32, tag="pp")
                for k in range(KC):
                    cs = ccat[:, k, blk * QB:(blk + 1) * QB]
                    nc.tensor.matmul(psk[:], wk_sb[:, k, :], cs,
                                     start=(k == 0), stop=(k == KC - 1))
                    nc.tensor.matmul(psv[:], wv_sb[:, k, :], cs,
                                     start=(k == 0), stop=(k == KC - 1))
                nc.vector.tensor_copy(KT[:, col0:col0 + QB], psk[:])
                nc.vector.tensor_copy(VT[:, col0:col0 + QB], psv[:])

            vtr_done = set()

            def vtr(b, mt0, mt1):
                # V^T via PE transpose into PSUM, then DVE copy into vg2.
                for mt in range(mt0, mt1):
                    g = b * MT + mt
                    mcol = b * N + mt * 128
                    for base, c0 in ((0, 0), (DH, DH + 1)):
                        pt = psP.tile([128, DH], BF16, tag="pp")
                        nc.tensor.transpose(
                            pt[:], VT[base:base + DH, mcol:mcol + 128],
                            ident[base:base + DH, base:base + DH])
                        nc.vector.tensor_copy(vg2[:, g, c0:c0 + DH], pt[:])
                    vtr_done.add(g)

            def attn_qb(b, qb, extras=None):
                if b not in ocats:
                    ocats[b] = opool.tile([128, N], BF16, tag="oc",
                                          name=f"ocat_b{b}_{_rep}")
                ocat = ocats[b]
                qcol = b * N + qb * QB
                oA = psO.tile([128, QB], F32, tag="oa")
                oB = psO.tile([128, QB], F32, tag="oa")
                sts = {}

                def emit_st(mc):
                    mcol = b * N + mc * 128
                    stAB = psSt.tile([128, 2 * QB], F32, tag="st")
                    nc.tensor.matmul(stAB[:, 0:QB],
                                     KT[0:DH, mcol:mcol + 128],
                                     QT[0:DH, qcol:qcol + QB],
                                     start=True, stop=True)
                    nc.tensor.matmul(stAB[:, QB:2 * QB],
                                     KT[DH:2 * DH, mcol:mcol + 128],
                                     QT[DH:2 * DH, qcol:qcol + QB],
                                     start=True, stop=True)
                    sts[mc] = stAB

                emit_st(0)
                for mc in range(MT):
                    g = b * MT + mc
                    stAB = sts.pop(mc)
                    eAB = epool.tile([128, 2 * QB], BF16, tag="e")
                    nc.scalar.activation(eAB[:], stAB[:], EXP, scale=SCALE)
                    if mc + 1 < MT:
                        emit_st(mc + 1)
                    if mc % 4 == 3 and extras:
                        extras.pop(0)()
                    last = (mc == MT - 1)
                    assert g in vtr_done, f"vg2 tile {g} used before emitted"
                    # rows 0..63 = O, row 64 = softmax denominator
                    nc.tensor.matmul(oA[0:DH + 1, :], vg2[:, g, 0:DH + 1],
                                     eAB[:, 0:QB], start=(mc == 0), stop=last)
                    nc.tensor.matmul(oB[0:DH + 1, :],
                                     vg2[:, g, DH + 1:2 * DH + 2],
                                     eAB[:, QB:2 * QB],
                                     start=(mc == 0), stop=last)
                for o_ps, row0 in ((oA, 0), (oB, DH)):
                    rr = bcp.tile([1, QB], F32, tag="rr")
                    nc.vector.reciprocal(rr[:], o_ps[DH:DH + 1, :])
                    bc = bcp.tile([DH, QB], F32, tag="bc")
                    nc.gpsimd.partition_broadcast(bc[:], rr[:])
                    nc.vector.tensor_mul(
                        ocat[row0:row0 + DH, qb * QB:(qb + 1) * QB],
                        o_ps[0:DH, :], bc[:])

            def wo_piece(b, nt):
                ocat = ocats[b]
                po1 = psP.tile([128, QB], F32, tag="pp")
                po2 = psP.tile([128, QB], F32, tag="pp")
                nc.tensor.matmul(po1[:], ocat[:, nt * 128:(nt + 1) * 128],
                                 wo_sb[:, 0:QB], start=True, stop=True)
                nc.tensor.matmul(po2[:], ocat[:, nt * 128:(nt + 1) * 128],
                                 wo_sb[:, QB:QDIM], start=True, stop=True)
                osb = outp.tile([128, QDIM], F32, tag="os")
                nc.vector.tensor_copy(osb[:, 0:QB], po1[:])
                nc.vector.tensor_copy(osb[:, QB:QDIM], po2[:])
                row0 = b * N + nt * 128
                nc.sync.dma_start(part[row0:row0 + 128, :], osb[:])

            # Interleaved schedule: ACT (exp) paces the attention stream;
            # everything else (next-batch projections, V transposes, Wo
            # matmuls, output DMA) is emitted into slots inside the
            # attention blocks to fill PE/DMA slack.
            def wo2(b, nt):
                return lambda: (wo_piece(b, nt), wo_piece(b, nt + 1))

            load_piece(0, "c")
            load_piece(0, "x")
            for blk in range(NQB):
                kv_piece(0, blk)
            q_piece(0, 0)
            vtr(0, 0, 4)

            attn_qb(0, 0, [lambda: vtr(0, 4, 8),
                           lambda: vtr(0, 8, 12),
                           lambda: vtr(0, 12, 16),
                           lambda: q_piece(0, 1)])
            attn_qb(0, 1, [lambda: q_piece(0, 2),
                           lambda: q_piece(0, 3),
                           lambda: load_piece(1, "c"),
                           wo2(0, 0)])
            attn_qb(0, 2, [wo2(0, 2),
                           lambda: kv_piece(1, 0),
                           lambda: kv_piece(1, 1),
                           lambda: load_piece(1, "x")])
            attn_qb(0, 3, [lambda: kv_piece(1, 2),
                           lambda: kv_piece(1, 3),
                           lambda: q_piece(1, 0),
                           lambda: vtr(1, 0, 8)])
            attn_qb(1, 0, [lambda: vtr(1, 8, 16),
                           lambda: q_piece(1, 1),
                           wo2(0, 4),
                           wo2(0, 6)])
            attn_qb(1, 1, [wo2(0, 8),
                           wo2(0, 10),
                           lambda: q_piece(1, 2),
                           wo2(0, 12)])
            attn_qb(1, 2, [wo2(0, 14),
                           lambda: q_piece(1, 3),
                           wo2(1, 0),
                           wo2(1, 2)])
            attn_qb(1, 3, [wo2(1, 4),
                           wo2(1, 6),
                           wo2(1, 8),
                           wo2(1, 10)])
            for nt in range(12, 16):
                wo_piece(1, nt)
    nc.compile()
    return nc


def make_in_maps(x, context, Wq, Wk, Wv, Wo):
    bf16 = mybir.dt.np(BF16)
    x = np.asarray(x, dtype=np.float32)
    context = np.asarray(context, dtype=np.float32)
    Wq = np.asarray(Wq, dtype=np.float32)
    Wk = np.asarray(Wk, dtype=np.float32)
    Wv = np.asarray(Wv, dtype=np.float32)
    Wo = np.asarray(Wo, dtype=np.float32)
    xT = np.ascontiguousarray(x.reshape(T, QDIM).T).astype(bf16)
    cT = np.ascontiguousarray(context.reshape(T, QDIM).T).astype(bf16)
    in_maps = []
    for c in range(NCORES):
        es = slice(c * ES, (c + 1) * ES)
        in_maps.append({
            "xT": xT,
            "cT": cT,
            "wqT": np.ascontiguousarray(Wq[es, :].T).astype(bf16),
            "wkT": np.ascontiguousarray(Wk[es, :].T).astype(bf16),
            "wvT": np.ascontiguousarray(Wv[es, :].T).astype(bf16),
            "woT": np.ascontiguousarray(Wo[:, es].T).astype(bf16),
        })
    return in_maps


_NC_CACHE = {}


def get_nc(reps: int = 1):
    if reps not in _NC_CACHE:
        _NC_CACHE[reps] = build_nc(reps)
    return _NC_CACHE[reps]


def run_on_hw(in_maps, reps: int = 1):
    nc = get_nc(reps)
    return run_bass_kernel_spmd(nc, in_maps, core_ids=list(range(NCORES)))


def kernel(x, context, Wq, Wk, Wv, Wo, bo):
    in_maps = make_in_maps(x, context, Wq, Wk, Wv, Wo)
    res = run_on_hw(in_maps, reps=1)
    acc = res.results[0]["part"].astype(np.float32).copy()
    for i in range(1, NCORES):
        acc += res.results[i]["part"]
    acc += np.asarray(bo, dtype=np.float32)[None, :]
    return acc.reshape(B, N, QDIM)


# revision 10
# speedup vs baseline: 1.6595x; 1.0695x over previous
"""CrossAttention kernel for 8 TRN2 NeuronCores (head-parallel sharding), v3.

Problem: x[2,2048,1024], context[2,2048,1024], 16 heads x 64 dim,
q/k/v projections + softmax attention + output projection.

Sharding: 2 heads per core (e-slice of 128 rows of Wq/Wk/Wv, 128 cols of Wo).
Each core computes a full-shape partial of the output projection for its
heads; the host sums the 8 partials and adds the bias.

Design notes:
- bf16 on-device datapath (host converts inputs); PSUM accumulation fp32.
- Inputs staged per batch into persistent SBUF tiles via 8 large HWDGE
  DMAs (512KB each) - minimizes SP-sequencer issue serialization.
- V^T tiles produced by one [128,128] DMA-transpose per key tile into
  vg2[ones | V_A | V_B | ones], so the softmax denominator folds into the
  attn*V matmuls as an extra lhsT column (output partition 0 / 64).
- Attention in 512-query blocks; both heads' S^T tiles land in one
  [128,1024] PSUM tile so exp runs as a single 1024-wide ACT instruction
  (ACT is the pacing engine at ~133us busy).
- The two heads' QK^T matmuls have 64-deep contraction and auto-place on
  disjoint PE row groups (tile_position (0,0)/(64,0)), overlapping on HW.
- Emission interleaves next-batch projections, Wo matmuls and output DMA
  into the ACT-paced attention stream.
"""
import sys

sys.path.insert(0, "/opt/trn_rl_repo")

import numpy as np
from contextlib import ExitStack

import concourse.bass as bass  # noqa: F401
import concourse.tile as tile
from concourse import bacc, mybir
from concourse.bass_utils import run_bass_kernel_spmd
from concourse.masks import make_identity

B, N, M = 2, 2048, 2048
QDIM = 1024
HEADS = 16
DH = 64
INNER = 1024
NCORES = 8
ES = INNER // NCORES        # 128: e-slice (2 heads * 64) per core
SCALE = DH ** -0.5
T = B * N                   # 4096 query tokens; key tokens likewise B*M
KC = QDIM // 128            # 8 contraction chunks for the projections
MT = M // 128               # 16 key tiles per batch
QB = 512                    # query block (1 PSUM bank of fp32)
NQB = N // QB               # 4 query blocks per batch
F32 = mybir.dt.float32
BF16 = mybir.dt.bfloat16
EXP = mybir.ActivationFunctionType.Exp


def build_nc(reps: int = 1):
    nc = bacc.Bacc("TRN2", target_bir_lowering=False, debug=False,
                   num_devices=NCORES)
    xT = nc.dram_tensor("xT", [QDIM, T], BF16, kind="ExternalInput").ap()
    cT = nc.dram_tensor("cT", [QDIM, T], BF16, kind="ExternalInput").ap()
    wqT = nc.dram_tensor("wqT", [QDIM, ES], BF16, kind="ExternalInput").ap()
    wkT = nc.dram_tensor("wkT", [QDIM, ES], BF16, kind="ExternalInput").ap()
    wvT = nc.dram_tensor("wvT", [QDIM, ES], BF16, kind="ExternalInput").ap()
    woT = nc.dram_tensor("woT", [ES, QDIM], BF16, kind="ExternalInput").ap()
    part = nc.dram_tensor("part", [T, QDIM], BF16, kind="ExternalOutput").ap()

    xT3 = xT.rearrange("(kc p) n -> kc p n", p=128)
    cT3 = cT.rearrange("(kc p) n -> kc p n", p=128)

    with tile.TileContext(nc) as tc, ExitStack() as ctx:
        const = ctx.enter_context(tc.tile_pool(name="const", bufs=1))
        big = ctx.enter_context(tc.tile_pool(name="bigsb", bufs=1))
        epool = ctx.enter_context(tc.tile_pool(name="epool", bufs=4))
        opool = ctx.enter_context(tc.tile_pool(name="opool", bufs=2))
        bcp = ctx.enter_context(tc.tile_pool(name="bcp", bufs=4))
        outp = ctx.enter_context(tc.tile_pool(name="outp", bufs=3))
        psO = ctx.enter_context(tc.tile_pool(name="psO", bufs=2, space="PSUM"))
        psSt = ctx.enter_context(tc.tile_pool(name="psSt", bufs=2, space="PSUM"))
        psP = ctx.enter_context(tc.tile_pool(name="psP", bufs=2, space="PSUM"))

        ident_f = const.tile([128, 128], F32)
        make_identity(nc, ident_f[:])
        ident = const.tile([128, 128], BF16)
        nc.vector.tensor_copy(ident[:], ident_f[:])
        wq_sb = const.tile([128, KC, ES], BF16)
        wk_sb = const.tile([128, KC, ES], BF16)
        wv_sb = const.tile([128, KC, ES], BF16)
        wo_sb = const.tile([128, QDIM], BF16)
        nc.gpsimd.dma_start(wq_sb[:], wqT.rearrange("(kc p) e -> p kc e", p=128))
        nc.gpsimd.dma_start(wk_sb[:], wkT.rearrange("(kc p) e -> p kc e", p=128))
        nc.gpsimd.dma_start(wv_sb[:], wvT.rearrange("(kc p) e -> p kc e", p=128))
        nc.gpsimd.dma_start(wo_sb[:], woT)

        for _rep in range(reps):
            QT = big.tile([128, T], BF16, tag="QT")
            KT = big.tile([128, T], BF16, tag="KT")
            VT = big.tile([128, T], BF16, tag="VT")
            # vg2 columns: [V_A (64) | ones | V_B (64) | ones]
            vg2 = big.tile([128, B * MT, 2 * DH + 2], BF16, tag="vg2")
            nc.vector.memset(vg2[:, :, DH], 1.0)
            nc.vector.memset(vg2[:, :, 2 * DH + 1], 1.0)
            ocats = {}
            xcats = {}

            def load_piece(b, which):
                src3, tag = (xT3, "xcat") if which == "x" else (cT3, "ccat")
                cat = big.tile([128, KC, N], BF16, tag=tag,
                               name=f"{tag}_{b}_{_rep}")
                for k in range(KC):
                    nc.sync.dma_start(cat[:, k, :], src3[k, :, b * N:(b + 1) * N])
                xcats[(b, which)] = cat

            def q_piece(b, blk):
                xcat = xcats[(b, "x")]
                col0 = b * N + blk * QB
                ps = psP.tile([128, QB], F32, tag="pp")
                for k in range(KC):
                    nc.tensor.matmul(ps[:], wq_sb[:, k, :],
                                     xcat[:, k, blk * QB:(blk + 1) * QB],
                                     start=(k == 0), stop=(k == KC - 1))
                nc.vector.tensor_copy(QT[:, col0:col0 + QB], ps[:])

            def kv_piece(b, blk):
                ccat = xcats[(b, "c")]
                col0 = b * N + blk * QB
                psk = psP.tile([128, QB], F32, tag="pp")
                psv = psP.tile([128, QB], F32, tag="pp")
                for k in range(KC):
                    cs = ccat[:, k, blk * QB:(blk + 1) * QB]
                    nc.tensor.matmul(psk[:], wk_sb[:, k, :], cs,
                                     start=(k == 0), stop=(k == KC - 1))
                    nc.tensor.matmul(psv[:], wv_sb[:, k, :], cs,
                                     start=(k == 0), stop=(k == KC - 1))
                nc.vector.tensor_copy(KT[:, col0:col0 + QB], psk[:])
                nc.vector.tensor_copy(VT[:, col0:col0 + QB], psv[:])

            vtr_done = set()

            def vtr(b, mt0, mt1):
                # V^T via PE transpose into PSUM, then DVE copy into vg2.
                for mt in range(mt0, mt1):
                    g = b * MT + mt
                    mcol = b * N + mt * 128
                    for base, c0 in ((0, 0), (DH, DH + 1)):
                        pt = psP.tile([128, DH], BF16, tag="pp")
                        nc.tensor.transpose(
                            pt[:], VT[base:base + DH, mcol:mcol + 128],
                            ident[base:base + DH, base:base + DH])
                        nc.vector.tensor_copy(vg2[:, g, c0:c0 + DH], pt[:])
                    vtr_done.add(g)

            def attn_qb(b, qb, extras=None):
                if b not in ocats:
                    ocats[b] = opool.tile([128, N], BF16, tag="oc",
                                          name=f"ocat_b{b}_{_rep}")
                ocat = ocats[b]
                qcol = b * N + qb * QB
                oA = psO.tile([128, QB], F32, tag="oa")
                oB = psO.tile([128, QB], F32, tag="oa")
                sts = {}

                def emit_st(mc):
                    mcol = b * N + mc * 128
                    stAB = psSt.tile([128, 2 * QB], F32, tag="st")
                    nc.tensor.matmul(stAB[:, 0:QB],
                                     KT[0:DH, mcol:mcol + 128],
                                     QT[0:DH, qcol:qcol + QB],
                                     start=True, stop=True)
                    nc.tensor.matmul(stAB[:, QB:2 * QB],
                                     KT[DH:2 * DH, mcol:mcol + 128],
                                     QT[DH:2 * DH, qcol:qcol + QB],
                                     start=True, stop=True)
                    sts[mc] = stAB

                emit_st(0)
                for mc in range(MT):
                    g = b * MT + mc
                    stAB = sts.pop(mc)
                    eAB = epool.tile([128, 2 * QB], BF16, tag="e")
                    nc.scalar.activation(eAB[:], stAB[:], EXP, scale=SCALE)
                    if mc + 1 < MT:
                        emit_st(mc + 1)
                    if mc % 4 == 3 and extras:
                        extras.pop(0)()
                    last = (mc == MT - 1)
                    assert g in vtr_done, f"vg2 tile {g} used before emitted"
                    # rows 0..63 = O, row 64 = softmax denominator
                    nc.tensor.matmul(oA[0:DH + 1, :], vg2[:, g, 0:DH + 1],
                                     eAB[:, 0:QB], start=(mc == 0), stop=last)
                    nc.tensor.matmul(oB[0:DH + 1, :],
                                     vg2[:, g, DH + 1:2 * DH + 2],
                                     eAB[:, QB:2 * QB],
                                     start=(mc == 0), stop=last)
                for o_ps, row0 in ((oA, 0), (oB, DH)):
                    rr = bcp.tile([1, QB], F32, tag="rr")
                    nc.vector.reciprocal(rr[:], o_ps[DH:DH + 1, :])
                    bc = bcp.tile([DH, QB], F32, tag="bc")
                    nc.gpsimd.partition_broadcast(bc[:], rr[:])
                    nc.vector.tensor_mul(
                        ocat[row0:row0 + DH, qb * QB:(qb + 1) * QB],
                        o_ps[0:DH, :], bc[:])

            def wo_piece(b, nt):
                ocat = ocats[b]
                po1 = psP.tile([128, QB], F32, tag="pp")
                po2 = psP.tile([128, QB], F32, tag="pp")
                nc.tensor.matmul(po1[:], ocat[:, nt * 128:(nt + 1) * 128],
                                 wo_sb[:, 0:QB], start=True, stop=True)
                nc.tensor.matmul(po2[:], ocat[:, nt * 128:(nt + 1) * 128],
                                 wo_sb[:, QB:QDIM], start=True, stop=True)
                osb = outp.tile([128, QDIM], BF16, tag="os")
                nc.vector.tensor_copy(osb[:, 0:QB], po1[:])
                nc.vector.tensor_copy(osb[:, QB:QDIM], po2[:])
                row0 = b * N + nt * 128
                nc.sync.dma_start(part[row0:row0 + 128, :], osb[:])

            # Interleaved schedule: ACT (exp) paces the attention stream;
            # everything else (next-batch projections, V transposes, Wo
            # matmuls, output DMA) is emitted into slots inside the
            # attention blocks to fill PE/DMA slack.
            def wo2(b, nt):
                return lambda: (wo_piece(b, nt), wo_piece(b, nt + 1))

            load_piece(0, "c")
            load_piece(0, "x")
            for blk in range(NQB):
                kv_piece(0, blk)
            q_piece(0, 0)
            vtr(0, 0, 4)

            attn_qb(0, 0, [lambda: vtr(0, 4, 8),
                           lambda: vtr(0, 8, 12),
                           lambda: vtr(0, 12, 16),
                           lambda: q_piece(0, 1)])
            attn_qb(0, 1, [lambda: q_piece(0, 2),
                           lambda: q_piece(0, 3),
                           lambda: load_piece(1, "c"),
                           wo2(0, 0)])
            attn_qb(0, 2, [wo2(0, 2),
                           lambda: kv_piece(1, 0),
                           lambda: kv_piece(1, 1),
                           lambda: load_piece(1, "x")])
            attn_qb(0, 3, [lambda: kv_piece(1, 2),
                           lambda: kv_piece(1, 3),
                           lambda: q_piece(1, 0),
                           lambda: vtr(1, 0, 8)])
            attn_qb(1, 0, [lambda: vtr(1, 8, 16),
                           lambda: q_piece(1, 1),
                           wo2(0, 4),
                           wo2(0, 6)])
            attn_qb(1, 1, [wo2(0, 8),
                           wo2(0, 10),
                           lambda: q_piece(1, 2),
                           wo2(0, 12)])
            attn_qb(1, 2, [wo2(0, 14),
                           lambda: q_piece(1, 3),
                           wo2(1, 0),
                           wo2(1, 2)])
            attn_qb(1, 3, [wo2(1, 4),
                           wo2(1, 6),
                           wo2(1, 8),
                           wo2(1, 10)])
            for nt in range(12, 16):
                wo_piece(1, nt)
    nc.compile()
    return nc


def make_in_maps(x, context, Wq, Wk, Wv, Wo):
    bf16 = mybir.dt.np(BF16)
    x = np.asarray(x, dtype=np.float32)
    context = np.asarray(context, dtype=np.float32)
    Wq = np.asarray(Wq, dtype=np.float32)
    Wk = np.asarray(Wk, dtype=np.float32)
    Wv = np.asarray(Wv, dtype=np.float32)
    Wo = np.asarray(Wo, dtype=np.float32)
    xT = np.ascontiguousarray(x.reshape(T, QDIM).T).astype(bf16)
    cT = np.ascontiguousarray(context.reshape(T, QDIM).T).astype(bf16)
    in_maps = []
    for c in range(NCORES):
        es = slice(c * ES, (c + 1) * ES)
        in_maps.append({
            "xT": xT,
            "cT": cT,
            "wqT": np.ascontiguousarray(Wq[es, :].T).astype(bf16),
            "wkT": np.ascontiguousarray(Wk[es, :].T).astype(bf16),
            "wvT": np.ascontiguousarray(Wv[es, :].T).astype(bf16),
            "woT": np.ascontiguousarray(Wo[:, es].T).astype(bf16),
        })
    return in_maps


_NC_CACHE = {}


def get_nc(reps: int = 1):
    if reps not in _NC_CACHE:
        _NC_CACHE[reps] = build_nc(reps)
    return _NC_CACHE[reps]


def run_on_hw(in_maps, reps: int = 1):
    nc = get_nc(reps)
    return run_bass_kernel_spmd(nc, in_maps, core_ids=list(range(NCORES)))


def kernel(x, context, Wq, Wk, Wv, Wo, bo):
    in_maps = make_in_maps(x, context, Wq, Wk, Wv, Wo)
    res = run_on_hw(in_maps, reps=1)
    acc = res.results[0]["part"].astype(np.float32).copy()
    for i in range(1, NCORES):
        acc += res.results[i]["part"]
    acc += np.asarray(bo, dtype=np.float32)[None, :]
    return acc.reshape(B, N, QDIM)


# revision 13
# speedup vs baseline: 1.7083x; 1.0294x over previous
"""CrossAttention kernel for 8 TRN2 NeuronCores (head-parallel sharding).

Measured: ~275us/rep on HW (queued-throughput slope), rel err ~3.1e-3.

Problem: x[2,2048,1024], context[2,2048,1024], 16 heads x 64 dim,
q/k/v projections + softmax attention + output projection.

Sharding: 2 heads per core (e-slice of 128 rows of Wq/Wk/Wv, 128 cols of Wo).
Each core computes a full-shape partial of the output projection for its
heads; the host sums the 8 partials and adds the bias.

Design notes:
- bf16 on-device datapath (host converts inputs); PSUM accumulation fp32.
- Inputs staged per batch into persistent SBUF tiles via 8 large HWDGE
  DMAs (512KB each) - minimizes SP-sequencer issue serialization.
- V^T tiles produced by one [128,128] DMA-transpose per key tile into
  vg2[ones | V_A | V_B | ones], so the softmax denominator folds into the
  attn*V matmuls as an extra lhsT column (output partition 0 / 64).
- Attention in 512-query blocks; both heads' S^T tiles land in one
  [128,1024] PSUM tile so exp runs as a single 1024-wide ACT instruction
  (ACT is the pacing engine at ~133us busy).
- The two heads' QK^T matmuls have 64-deep contraction and auto-place on
  disjoint PE row groups (tile_position (0,0)/(64,0)), overlapping on HW.
- Emission interleaves next-batch projections, Wo matmuls and output DMA
  into the ACT-paced attention stream.
"""
import sys

sys.path.insert(0, "/opt/trn_rl_repo")

import numpy as np
from contextlib import ExitStack

import concourse.bass as bass  # noqa: F401
import concourse.tile as tile
from concourse import bacc, mybir
from concourse.bass_utils import run_bass_kernel_spmd
from concourse.masks import make_identity

B, N, M = 2, 2048, 2048
QDIM = 1024
HEADS = 16
DH = 64
INNER = 1024
NCORES = 8
ES = INNER // NCORES        # 128: e-slice (2 heads * 64) per core
SCALE = DH ** -0.5
T = B * N                   # 4096 query tokens; key tokens likewise B*M
KC = QDIM // 128            # 8 contraction chunks for the projections
MT = M // 128               # 16 key tiles per batch
QB = 512                    # query block (1 PSUM bank of fp32)
NQB = N // QB               # 4 query blocks per batch
F32 = mybir.dt.float32
BF16 = mybir.dt.bfloat16
EXP = mybir.ActivationFunctionType.Exp


def build_nc(reps: int = 1):
    nc = bacc.Bacc("TRN2", target_bir_lowering=False, debug=False,
                   num_devices=NCORES)
    xT = nc.dram_tensor("xT", [QDIM, T], BF16, kind="ExternalInput").ap()
    cT = nc.dram_tensor("cT", [QDIM, T], BF16, kind="ExternalInput").ap()
    wqT = nc.dram_tensor("wqT", [QDIM, ES], BF16, kind="ExternalInput").ap()
    wkT = nc.dram_tensor("wkT", [QDIM, ES], BF16, kind="ExternalInput").ap()
    wvT = nc.dram_tensor("wvT", [QDIM, ES], BF16, kind="ExternalInput").ap()
    woT = nc.dram_tensor("woT", [ES, QDIM], BF16, kind="ExternalInput").ap()
    part = nc.dram_tensor("part", [T, QDIM], BF16, kind="ExternalOutput").ap()

    xT3 = xT.rearrange("(kc p) n -> kc p n", p=128)
    cT3 = cT.rearrange("(kc p) n -> kc p n", p=128)

    with tile.TileContext(nc) as tc, ExitStack() as ctx:
        const = ctx.enter_context(tc.tile_pool(name="const", bufs=1))
        big = ctx.enter_context(tc.tile_pool(name="bigsb", bufs=1))
        epool = ctx.enter_context(tc.tile_pool(name="epool", bufs=4))
        opool = ctx.enter_context(tc.tile_pool(name="opool", bufs=2))
        bcp = ctx.enter_context(tc.tile_pool(name="bcp", bufs=4))
        outp = ctx.enter_context(tc.tile_pool(name="outp", bufs=3))
        psO = ctx.enter_context(tc.tile_pool(name="psO", bufs=2, space="PSUM"))
        psSt = ctx.enter_context(tc.tile_pool(name="psSt", bufs=2, space="PSUM"))
        psP = ctx.enter_context(tc.tile_pool(name="psP", bufs=2, space="PSUM"))

        ident_f = const.tile([128, 128], F32)
        make_identity(nc, ident_f[:])
        ident = const.tile([128, 128], BF16)
        nc.vector.tensor_copy(ident[:], ident_f[:])
        wq_sb = const.tile([128, KC, ES], BF16)
        wk_sb = const.tile([128, KC, ES], BF16)
        wv_sb = const.tile([128, KC, ES], BF16)
        wo_sb = const.tile([128, QDIM], BF16)
        nc.gpsimd.dma_start(wq_sb[:], wqT.rearrange("(kc p) e -> p kc e", p=128))
        nc.gpsimd.dma_start(wk_sb[:], wkT.rearrange("(kc p) e -> p kc e", p=128))
        nc.gpsimd.dma_start(wv_sb[:], wvT.rearrange("(kc p) e -> p kc e", p=128))
        nc.gpsimd.dma_start(wo_sb[:], woT)

        for _rep in range(reps):
            # rep-parity tags double-buffer the persistent tiles so
            # back-to-back invocations pipeline (rep r+1's projections
            # don't wait for rep r's final attention reads).
            par = _rep % 2
            QT = big.tile([128, T], BF16, tag=f"QT{par}")
            KT = big.tile([128, T], BF16, tag=f"KT{par}")
            VT = big.tile([128, T], BF16, tag=f"VT{par}")
            # vg2 columns: [V_A (64) | ones | V_B (64) | ones]
            vg2 = big.tile([128, B * MT, 2 * DH + 2], BF16, tag=f"vg2{par}")
            nc.vector.memset(vg2[:, :, DH], 1.0)
            nc.vector.memset(vg2[:, :, 2 * DH + 1], 1.0)
            ocats = {}
            xcats = {}

            def load_piece(b, which):
                src3, tag = (xT3, "xcat") if which == "x" else (cT3, "ccat")
                cat = big.tile([128, KC, N], BF16, tag=tag,
                               name=f"{tag}_{b}_{_rep}")
                for k in range(KC):
                    nc.sync.dma_start(cat[:, k, :], src3[k, :, b * N:(b + 1) * N])
                xcats[(b, which)] = cat

            def q_piece(b, blk):
                xcat = xcats[(b, "x")]
                col0 = b * N + blk * QB
                ps = psP.tile([128, QB], F32, tag="pp")
                for k in range(KC):
                    nc.tensor.matmul(ps[:], wq_sb[:, k, :],
                                     xcat[:, k, blk * QB:(blk + 1) * QB],
                                     start=(k == 0), stop=(k == KC - 1))
                nc.vector.tensor_copy(QT[:, col0:col0 + QB], ps[:])

            def kv_piece(b, blk):
                ccat = xcats[(b, "c")]
                col0 = b * N + blk * QB
                psk = psP.tile([128, QB], F32, tag="pp")
                psv = psP.tile([128, QB], F32, tag="pp")
                for k in range(KC):
                    cs = ccat[:, k, blk * QB:(blk + 1) * QB]
                    nc.tensor.matmul(psk[:], wk_sb[:, k, :], cs,
                                     start=(k == 0), stop=(k == KC - 1))
                    nc.tensor.matmul(psv[:], wv_sb[:, k, :], cs,
                                     start=(k == 0), stop=(k == KC - 1))
                nc.vector.tensor_copy(KT[:, col0:col0 + QB], psk[:])
                nc.vector.tensor_copy(VT[:, col0:col0 + QB], psv[:])

            vtr_done = set()

            def vtr(b, mt0, mt1):
                # V^T via PE transpose into PSUM, then DVE copy into vg2.
                for mt in range(mt0, mt1):
                    g = b * MT + mt
                    mcol = b * N + mt * 128
                    for base, c0 in ((0, 0), (DH, DH + 1)):
                        pt = psP.tile([128, DH], BF16, tag="pp")
                        nc.tensor.transpose(
                            pt[:], VT[base:base + DH, mcol:mcol + 128],
                            ident[base:base + DH, base:base + DH])
                        nc.vector.tensor_copy(vg2[:, g, c0:c0 + DH], pt[:])
                    vtr_done.add(g)

            def attn_qb(b, qb, extras=None):
                if b not in ocats:
                    ocats[b] = opool.tile([128, N], BF16, tag=f"oc{par}",
                                          name=f"ocat_b{b}_{_rep}")
                ocat = ocats[b]
                qcol = b * N + qb * QB
                oA = psO.tile([128, QB], F32, tag="oa")
                oB = psO.tile([128, QB], F32, tag="oa")
                sts = {}

                def emit_st(mc):
                    mcol = b * N + mc * 128
                    stAB = psSt.tile([128, 2 * QB], F32, tag="st")
                    nc.tensor.matmul(stAB[:, 0:QB],
                                     KT[0:DH, mcol:mcol + 128],
                                     QT[0:DH, qcol:qcol + QB],
                                     start=True, stop=True)
                    nc.tensor.matmul(stAB[:, QB:2 * QB],
                                     KT[DH:2 * DH, mcol:mcol + 128],
                                     QT[DH:2 * DH, qcol:qcol + QB],
                                     start=True, stop=True)
                    sts[mc] = stAB

                emit_st(0)
                for mc in range(MT):
                    g = b * MT + mc
                    stAB = sts.pop(mc)
                    eAB = epool.tile([128, 2 * QB], BF16, tag="e")
                    nc.scalar.activation(eAB[:], stAB[:], EXP, scale=SCALE)
                    if mc + 1 < MT:
                        emit_st(mc + 1)
                    if mc % 4 == 3 and extras:
                        extras.pop(0)()
                    last = (mc == MT - 1)
                    assert g in vtr_done, f"vg2 tile {g} used before emitted"
                    # rows 0..63 = O, row 64 = softmax denominator
                    nc.tensor.matmul(oA[0:DH + 1, :], vg2[:, g, 0:DH + 1],
                                     eAB[:, 0:QB], start=(mc == 0), stop=last)
                    nc.tensor.matmul(oB[0:DH + 1, :],
                                     vg2[:, g, DH + 1:2 * DH + 2],
                                     eAB[:, QB:2 * QB],
                                     start=(mc == 0), stop=last)
                for o_ps, row0 in ((oA, 0), (oB, DH)):
                    rr = bcp.tile([1, QB], F32, tag="rr")
                    nc.vector.reciprocal(rr[:], o_ps[DH:DH + 1, :])
                    bc = bcp.tile([DH, QB], F32, tag="bc")
                    nc.gpsimd.partition_broadcast(bc[:], rr[:])
                    nc.vector.tensor_mul(
                        ocat[row0:row0 + DH, qb * QB:(qb + 1) * QB],
                        o_ps[0:DH, :], bc[:])

            def wo_piece(b, nt):
                ocat = ocats[b]
                po1 = psP.tile([128, QB], F32, tag="pp")
                po2 = psP.tile([128, QB], F32, tag="pp")
                nc.tensor.matmul(po1[:], ocat[:, nt * 128:(nt + 1) * 128],
                                 wo_sb[:, 0:QB], start=True, stop=True)
                nc.tensor.matmul(po2[:], ocat[:, nt * 128:(nt + 1) * 128],
                                 wo_sb[:, QB:QDIM], start=True, stop=True)
                osb = outp.tile([128, QDIM], BF16, tag="os")
                nc.vector.tensor_copy(osb[:, 0:QB], po1[:])
                nc.vector.tensor_copy(osb[:, QB:QDIM], po2[:])
                row0 = b * N + nt * 128
                nc.sync.dma_start(part[row0:row0 + 128, :], osb[:])

            # Interleaved schedule: ACT (exp) paces the attention stream;
            # everything else (next-batch projections, V transposes, Wo
            # matmuls, output DMA) is emitted into slots inside the
            # attention blocks to fill PE/DMA slack.
            def wo2(b, nt):
                return lambda: (wo_piece(b, nt), wo_piece(b, nt + 1))

            load_piece(0, "c")
            load_piece(0, "x")
            for blk in range(NQB):
                kv_piece(0, blk)
            q_piece(0, 0)
            vtr(0, 0, 4)

            attn_qb(0, 0, [lambda: vtr(0, 4, 8),
                           lambda: vtr(0, 8, 12),
                           lambda: vtr(0, 12, 16),
                           lambda: q_piece(0, 1)])
            attn_qb(0, 1, [lambda: q_piece(0, 2),
                           lambda: q_piece(0, 3),
                           lambda: load_piece(1, "c"),
                           wo2(0, 0)])
            attn_qb(0, 2, [wo2(0, 2),
                           lambda: kv_piece(1, 0),
                           lambda: kv_piece(1, 1),
                           lambda: load_piece(1, "x")])
            attn_qb(0, 3, [lambda: kv_piece(1, 2),
                           lambda: kv_piece(1, 3),
                           lambda: q_piece(1, 0),
                           lambda: vtr(1, 0, 8)])
            attn_qb(1, 0, [lambda: vtr(1, 8, 16),
                           lambda: q_piece(1, 1),
                           wo2(0, 4),
                           wo2(0, 6)])
            attn_qb(1, 1, [wo2(0, 8),
                           wo2(0, 10),
                           lambda: q_piece(1, 2),
                           wo2(0, 12)])
            attn_qb(1, 2, [wo2(0, 14),
                           lambda: q_piece(1, 3),
                           wo2(1, 0),
                           wo2(1, 2)])
            attn_qb(1, 3, [wo2(1, 4),
                           wo2(1, 6),
                           wo2(1, 8),
                           wo2(1, 10)])
            for nt in range(12, 16):
                wo_piece(1, nt)
    nc.compile()
    return nc


def make_in_maps(x, context, Wq, Wk, Wv, Wo):
    bf16 = mybir.dt.np(BF16)
    x = np.asarray(x, dtype=np.float32)
    context = np.asarray(context, dtype=np.float32)
    Wq = np.asarray(Wq, dtype=np.float32)
    Wk = np.asarray(Wk, dtype=np.float32)
    Wv = np.asarray(Wv, dtype=np.float32)
    Wo = np.asarray(Wo, dtype=np.float32)
    xT = np.ascontiguousarray(x.reshape(T, QDIM).T).astype(bf16)
    cT = np.ascontiguousarray(context.reshape(T, QDIM).T).astype(bf16)
    in_maps = []
    for c in range(NCORES):
        es = slice(c * ES, (c + 1) * ES)
        in_maps.append({
            "xT": xT,
            "cT": cT,
            "wqT": np.ascontiguousarray(Wq[es, :].T).astype(bf16),
            "wkT": np.ascontiguousarray(Wk[es, :].T).astype(bf16),
            "wvT": np.ascontiguousarray(Wv[es, :].T).astype(bf16),
            "woT": np.ascontiguousarray(Wo[:, es].T).astype(bf16),
        })
    return in_maps


_NC_CACHE = {}


def get_nc(reps: int = 1):
    if reps not in _NC_CACHE:
        _NC_CACHE[reps] = build_nc(reps)
    return _NC_CACHE[reps]


def run_on_hw(in_maps, reps: int = 1):
    nc = get_nc(reps)
    return run_bass_kernel_spmd(nc, in_maps, core_ids=list(range(NCORES)))


def kernel(x, context, Wq, Wk, Wv, Wo, bo):
    in_maps = make_in_maps(x, context, Wq, Wk, Wv, Wo)
    res = run_on_hw(in_maps, reps=1)
    acc = res.results[0]["part"].astype(np.float32).copy()
    for i in range(1, NCORES):
        acc += res.results[i]["part"]
    acc += np.asarray(bo, dtype=np.float32)[None, :]
    return acc.reshape(B, N, QDIM)
